# revision 7
# baseline (speedup 1.0000x reference)
"""Multi-head attention (B=4, S=2048, M=1024, H=16, D=64) on 8 trn2 cores.

Sharding: core c = (b, g) with b = c // 2 (batch), g = c % 2 (head group of 8
heads).  Each core computes q/k/v projections for its 8 heads, causal
attention, and a partial output projection (contraction over its 512 feature
rows of Wo).  Host sums the two partials per batch and adds the bias.

Device-side layouts (all fp32):
  xqT/xkT/xvT  [1024(m), 2048(s)]   host-transposed activations
  wq/wk/wv     [1024(m), 512(dh)]   dh = 64*h_local + d  (head-major)
  wo           [512(dh), 1024(n)]
  qT,kT        [512(dh), 2048(s)]   = (X W)^T, computed as W.T @ X.T
  v            [2048(s), 520]       per 128-row tile: cols 65h..65h+63 = v_h,
                                    col 65h+64 = 1.0 (softmax denominator)
  scoresT      [sk, sq]             = kT_h.T @ qT_h  (contraction over d=64)
  PT           exp((scoresT + mask)/8)  -- no max subtraction (|scores/8|<~2)
  outT_h       [65, sq]             = [v_h|1].T @ PT ; row 64 = sum_sk PT
  attnT        [512(dh), sq]        = outT_h / denom, heads stacked
  out_partial  [2048(s), 1024(n)]   = attnT.T @ wo   (no bias)
"""

import os
import sys

for _p in ("/opt/trn_rl_repo", "/root/.axon_site/_ro/trn_rl_repo"):
    if os.path.isdir(_p) and _p not in sys.path:
        sys.path.append(_p)

import numpy as np

B, S, M, H, D = 4, 2048, 1024, 16, 64
G = 2               # head groups (cores per batch)
HPG = H // G        # heads per group = 8
DH = HPG * D        # feature rows per group = 512
NCORES = B * G
SB = 512            # sq block (matmul N)
CK = 128            # sk chunk (matmul M / partition)
NJ = S // SB        # 4 sq blocks
NC = S // CK        # 16 sk chunks
MK = M // 128       # 8 m chunks

_PROG_CACHE = {}


def _build_program(variant):
    """variant: 'causal' | 'allones' | 'general'"""
    import concourse.bass as bass
    import concourse.bacc as bacc
    import concourse.mybir as mybir
    from concourse import tile
    from contextlib import ExitStack

    f32 = mybir.dt.float32
    nc = bacc.Bacc("TRN2", target_bir_lowering=False, debug=False, num_devices=NCORES)

    xqT = nc.dram_tensor("xqT", [M, S], f32, kind="ExternalInput").ap()
    xkT = nc.dram_tensor("xkT", [M, S], f32, kind="ExternalInput").ap()
    xvT = nc.dram_tensor("xvT", [M, S], f32, kind="ExternalInput").ap()
    wq = nc.dram_tensor("wq", [M, DH], f32, kind="ExternalInput").ap()
    wk = nc.dram_tensor("wk", [M, DH], f32, kind="ExternalInput").ap()
    wv = nc.dram_tensor("wv", [M, DH], f32, kind="ExternalInput").ap()
    wo = nc.dram_tensor("wo", [DH, M], f32, kind="ExternalInput").ap()
    tri = nc.dram_tensor("tri", [128, 128], f32, kind="ExternalInput").ap()
    if variant == "general":
        maskT = nc.dram_tensor("maskT", [S, S], f32, kind="ExternalInput").ap()
    out = nc.dram_tensor("out", [S, M], f32, kind="ExternalOutput").ap()

    with tile.TileContext(nc) as tc, ExitStack() as ctx:
        ep = ctx.enter_context

        w_pool = ep(tc.tile_pool(name="w", bufs=9))
        x_pool = ep(tc.tile_pool(name="x", bufs=9))
        wo_pool = ep(tc.tile_pool(name="wo", bufs=1))
        qT_pool = ep(tc.tile_pool(name="qT", bufs=1))
        kT_pool = ep(tc.tile_pool(name="kT", bufs=1))
        v_pool = ep(tc.tile_pool(name="v", bufs=1))
        pt_pool = ep(tc.tile_pool(name="pt", bufs=4))
        at_pool = ep(tc.tile_pool(name="at", bufs=8))
        nrm_pool = ep(tc.tile_pool(name="nrm", bufs=2))
        out_pool = ep(tc.tile_pool(name="outp", bufs=4))
        misc_pool = ep(tc.tile_pool(name="misc", bufs=1))
        if variant == "general":
            mk_pool = ep(tc.tile_pool(name="mk", bufs=4))

        ps_mm = ep(tc.tile_pool(name="ps_mm", bufs=2, space="PSUM"))
        ps_sc = ep(tc.tile_pool(name="ps_sc", bufs=2, space="PSUM"))
        ps_pv = ep(tc.tile_pool(name="ps_pv", bufs=2, space="PSUM"))
        ps_rb = ep(tc.tile_pool(name="ps_rb", bufs=1, space="PSUM"))

        # constants
        tri_sb = misc_pool.tile([128, 128], f32, name="tri_sb")
        nc.gpsimd.dma_start(tri_sb[:], tri[:])
        ones1_sb = misc_pool.tile([1, 64], f32, name="ones1_sb")
        nc.gpsimd.memset(ones1_sb[:], 1.0)

        # ---- phase 1: projections ----
        qT_sb = [qT_pool.tile([128, S], f32, name=f"qT{d}") for d in range(4)]
        kT_sb = [kT_pool.tile([128, S], f32, name=f"kT{d}") for d in range(4)]
        v_sb = [v_pool.tile([128, HPG * 65], f32, name=f"v{t}") for t in range(NC)]

        for t in range(NC):
            # ones columns for the softmax denominator
            v3 = v_sb[t].rearrange("p (h c) -> p h c", h=HPG, c=65)
            nc.gpsimd.memset(v3[:, :, 64:65], 1.0)

        for w_dram, x_dram, kind in ((wq, xqT, "q"), (wk, xkT, "k"), (wv, xvT, "v")):
            w_ch = []
            for mc in range(MK):
                wt = w_pool.tile([128, DH], f32, name=f"w_{kind}{mc}", tag="w")
                nc.gpsimd.dma_start(wt[:], w_dram[mc * 128:(mc + 1) * 128, :])
                w_ch.append(wt)
            for j in range(NJ):
                x_ch = []
                for mc in range(MK):
                    xt = x_pool.tile([128, SB], f32, name=f"x_{kind}{j}_{mc}", tag="x")
                    nc.gpsimd.dma_start(
                        xt[:], x_dram[mc * 128:(mc + 1) * 128, j * SB:(j + 1) * SB])
                    x_ch.append(xt)
                if kind in ("q", "k"):
                    dst = qT_sb if kind == "q" else kT_sb
                    for d in range(4):
                        ps = ps_mm.tile([128, SB], f32, name=f"ps_{kind}{j}_{d}", tag="mm")
                        for mc in range(MK):
                            nc.tensor.matmul(
                                ps[:], w_ch[mc][:, d * 128:(d + 1) * 128], x_ch[mc][:],
                                start=(mc == 0), stop=(mc == MK - 1))
                        nc.vector.tensor_copy(dst[d][:, j * SB:(j + 1) * SB], ps[:])
                else:
                    for st in range(4):
                        t = 4 * j + st
                        ps = ps_mm.tile([128, DH], f32, name=f"ps_v{t}", tag="mm")
                        for mc in range(MK):
                            nc.tensor.matmul(
                                ps[:], x_ch[mc][:, st * 128:(st + 1) * 128], w_ch[mc][:],
                                start=(mc == 0), stop=(mc == MK - 1))
                        v3 = v_sb[t].rearrange("p (h c) -> p h c", h=HPG, c=65)
                        p3 = ps.rearrange("p (h c) -> p h c", h=HPG, c=64)
                        nc.vector.tensor_copy(v3[:, :, 0:64], p3[:])

        wo_sb = []
        for d in range(4):
            wt = wo_pool.tile([128, M], f32, name=f"wo{d}")
            nc.gpsimd.dma_start(wt[:], wo[d * 128:(d + 1) * 128, :])
            wo_sb.append(wt)

        # ---- phase 2: attention + output projection, per sq block ----
        for j in range(NJ):
            nchunks = 4 * (j + 1) if variant == "causal" else NC
            at_tiles = []
            for h in range(HPG):
                dtile, drow = h // 2, 64 * (h % 2)
                pv = ps_pv.tile([65, SB], f32, name=f"pv{j}_{h}", tag="pv")
                for c in range(nchunks):
                    sc = ps_sc.tile([128, SB], f32, name=f"sc{j}_{h}_{c}", tag="sc")
                    nc.tensor.matmul(
                        sc[:],
                        kT_sb[dtile][drow:drow + 64, c * CK:(c + 1) * CK],
                        qT_sb[dtile][drow:drow + 64, j * SB:(j + 1) * SB],
                        start=True, stop=True)
                    pt = pt_pool.tile([128, SB], f32, name=f"pt{j}_{h}_{c}", tag="pt")
                    if variant == "causal" and c >= 4 * j:
                        o = 128 * (c - 4 * j)
                        nc.vector.tensor_add(
                            sc[:, o:o + 128], sc[:, o:o + 128], tri_sb[:])
                        if o > 0:
                            nc.gpsimd.memset(pt[:, 0:o], 0.0)
                        nc.scalar.activation(
                            pt[:, o:SB], sc[:, o:SB],
                            mybir.ActivationFunctionType.Exp, scale=0.125)
                    else:
                        if variant == "general":
                            mk = mk_pool.tile([128, SB], f32, name=f"mk{j}_{h}_{c}", tag="mk")
                            nc.gpsimd.dma_start(
                                mk[:],
                                maskT[c * CK:(c + 1) * CK, j * SB:(j + 1) * SB])
                            nc.vector.tensor_add(sc[:], sc[:], mk[:])
                        nc.scalar.activation(
                            pt[:], sc[:],
                            mybir.ActivationFunctionType.Exp, scale=0.125)
                    nc.tensor.matmul(
                        pv[:], v_sb[c][:, 65 * h:65 * h + 65], pt[:],
                        start=(c == 0), stop=(c == nchunks - 1))
                dn = nrm_pool.tile([1, SB], f32, name=f"dn{j}_{h}", tag="dn")
                nc.vector.tensor_copy(dn[:], pv[64:65, :])
                dnr = nrm_pool.tile([1, SB], f32, name=f"dnr{j}_{h}", tag="dnr")
                nc.vector.reciprocal(dnr[:], dn[:])
                rb = ps_rb.tile([64, SB], f32, name=f"rb{j}_{h}", tag="rb")
                nc.tensor.matmul(rb[:], ones1_sb[:], dnr[:], start=True, stop=True)
                rb_sb = nrm_pool.tile([64, SB], f32, name=f"rbs{j}_{h}", tag="rbs")
                nc.scalar.copy(rb_sb[:], rb[:])
                if h % 2 == 0:
                    at = at_pool.tile([128, SB], f32, name=f"at{j}_{h // 2}", tag="at")
                    at_tiles.append(at)
                else:
                    at = at_tiles[-1]
                nc.vector.tensor_mul(
                    at[drow:drow + 64, :], pv[0:64, :], rb_sb[:])

            for ss in range(4):
                for nh in range(2):
                    ps = ps_mm.tile([128, SB], f32, name=f"po{j}_{ss}_{nh}", tag="mm")
                    for d in range(4):
                        nc.tensor.matmul(
                            ps[:],
                            at_tiles[d][:, ss * 128:(ss + 1) * 128],
                            wo_sb[d][:, nh * SB:(nh + 1) * SB],
                            start=(d == 0), stop=(d == 3))
                    ot = out_pool.tile([128, SB], f32, name=f"ot{j}_{ss}_{nh}", tag="ot")
                    nc.vector.tensor_copy(ot[:], ps[:])
                    r0 = j * SB + ss * 128
                    nc.gpsimd.dma_start(
                        out[r0:r0 + 128, nh * SB:(nh + 1) * SB], ot[:])

    nc.compile()
    return nc


def _get_program(variant):
    if variant not in _PROG_CACHE:
        _PROG_CACHE[variant] = _build_program(variant)
    return _PROG_CACHE[variant]


def _host_prep(queries, keys, values, masks, Wq, Wk, Wv):
    """Build the 8 per-core input maps."""
    tril = np.tril(np.ones((S, S), dtype=bool))
    if all(np.array_equal(masks[b], tril) for b in range(B)):
        variant = "causal"
    elif masks.all():
        variant = "allones"
    else:
        variant = "general"

    sq = np.arange(128)
    tri_np = np.where(sq[None, :] >= sq[:, None], 0.0, -1.0e6).astype(np.float32)

    # [H, M, D] -> [M, H*D] head-major per group
    def wcat(w, g):
        return np.ascontiguousarray(
            w[g * HPG:(g + 1) * HPG].transpose(1, 0, 2).reshape(M, DH))

    in_maps = []
    for c in range(NCORES):
        b, g = c // G, c % G
        m = {
            "xqT": np.ascontiguousarray(queries[b].T),
            "xkT": np.ascontiguousarray(keys[b].T),
            "xvT": np.ascontiguousarray(values[b].T),
            "wq": wcat(Wq, g),
            "wk": wcat(Wk, g),
            "wv": wcat(Wv, g),
            "tri": tri_np,
        }
        if variant == "general":
            m["maskT"] = np.where(masks[b].T, 0.0, -1.0e6).astype(np.float32)
        in_maps.append(m)
    return variant, in_maps


def run(queries, keys, values, masks, Wq, Wk, Wv, Wo, bo, trace=False):
    from concourse import bass_utils

    queries = np.asarray(queries, np.float32)
    keys = np.asarray(keys, np.float32)
    values = np.asarray(values, np.float32)
    masks = np.asarray(masks, bool)
    Wq = np.asarray(Wq, np.float32)
    Wk = np.asarray(Wk, np.float32)
    Wv = np.asarray(Wv, np.float32)
    Wo = np.asarray(Wo, np.float32)
    bo = np.asarray(bo, np.float32)

    variant, in_maps = _host_prep(queries, keys, values, masks, Wq, Wk, Wv)
    for c in range(NCORES):
        g = c % G
        in_maps[c]["wo"] = np.ascontiguousarray(Wo[g * DH:(g + 1) * DH, :])

    nc = _get_program(variant)
    res = bass_utils.run_bass_kernel_spmd(
        nc, in_maps, list(range(NCORES)), trace=trace)

    out = np.empty((B, S, M), np.float32)
    for b in range(B):
        out[b] = res.results[G * b]["out"] + res.results[G * b + 1]["out"] + bo
    return out, res


def kernel(queries, keys, values, masks, Wq, Wk, Wv, Wo, bo):
    out, _ = run(queries, keys, values, masks, Wq, Wk, Wv, Wo, bo, trace=False)
    return out


# revision 8
# speedup vs baseline: 1.0458x; 1.0458x over previous
"""Multi-head attention (B=4, S=2048, M=1024, H=16, D=64) on 8 trn2 cores.

Sharding: core c = (b, g) with b = c // 2 (batch), g = c % 2 (head group of 8
heads).  Each core computes q/k/v projections for its 8 heads, causal
attention, and a partial output projection (contraction over its 512 feature
rows of Wo).  Host sums the two partials per batch and adds the bias.

Device-side layouts (all fp32):
  xqT/xkT/xvT  [1024(m), 2048(s)]   host-transposed activations
  wq/wk/wv     [1024(m), 512(dh)]   dh = 64*h_local + d  (head-major)
  wo           [512(dh), 1024(n)]
  qT,kT        [512(dh), 2048(s)]   = (X W)^T, computed as W.T @ X.T
  v            [2048(s), 520]       per 128-row tile: cols 65h..65h+63 = v_h,
                                    col 65h+64 = 1.0 (softmax denominator)
  scoresT      [sk, sq]             = kT_h.T @ qT_h  (contraction over d=64)
  PT           exp((scoresT + mask)/8)  -- no max subtraction (|scores/8|<~2)
  outT_h       [65, sq]             = [v_h|1].T @ PT ; row 64 = sum_sk PT
  attnT        [512(dh), sq]        = outT_h / denom, heads stacked
  out_partial  [2048(s), 1024(n)]   = attnT.T @ wo   (no bias)
"""

import os
import sys

for _p in ("/opt/trn_rl_repo", "/root/.axon_site/_ro/trn_rl_repo"):
    if os.path.isdir(_p) and _p not in sys.path:
        sys.path.append(_p)

import numpy as np

B, S, M, H, D = 4, 2048, 1024, 16, 64
G = 2               # head groups (cores per batch)
HPG = H // G        # heads per group = 8
DH = HPG * D        # feature rows per group = 512
NCORES = B * G
SB = 512            # sq block (matmul N)
CK = 128            # sk chunk (matmul M / partition)
NJ = S // SB        # 4 sq blocks
NC = S // CK        # 16 sk chunks
MK = M // 128       # 8 m chunks

_PROG_CACHE = {}


def _build_program(variant):
    """variant: 'causal' | 'allones' | 'general'"""
    import concourse.bass as bass
    import concourse.bacc as bacc
    import concourse.mybir as mybir
    from concourse import tile
    from contextlib import ExitStack

    f32 = mybir.dt.float32
    nc = bacc.Bacc("TRN2", target_bir_lowering=False, debug=False, num_devices=NCORES)

    xqT = nc.dram_tensor("xqT", [M, S], f32, kind="ExternalInput").ap()
    xkT = nc.dram_tensor("xkT", [M, S], f32, kind="ExternalInput").ap()
    xvT = nc.dram_tensor("xvT", [M, S], f32, kind="ExternalInput").ap()
    wq = nc.dram_tensor("wq", [M, DH], f32, kind="ExternalInput").ap()
    wk = nc.dram_tensor("wk", [M, DH], f32, kind="ExternalInput").ap()
    wv = nc.dram_tensor("wv", [M, DH], f32, kind="ExternalInput").ap()
    wo = nc.dram_tensor("wo", [DH, M], f32, kind="ExternalInput").ap()
    tri = nc.dram_tensor("tri", [128, 128], f32, kind="ExternalInput").ap()
    if variant == "general":
        maskT = nc.dram_tensor("maskT", [S, S], f32, kind="ExternalInput").ap()
    out = nc.dram_tensor("out", [S, M], f32, kind="ExternalOutput").ap()

    with tile.TileContext(nc) as tc, ExitStack() as ctx:
        ep = ctx.enter_context

        w_pool = ep(tc.tile_pool(name="w", bufs=9))
        x_pool = ep(tc.tile_pool(name="x", bufs=10))
        wo_pool = ep(tc.tile_pool(name="wo", bufs=1))
        qT_pool = ep(tc.tile_pool(name="qT", bufs=1))
        kT_pool = ep(tc.tile_pool(name="kT", bufs=1))
        v_pool = ep(tc.tile_pool(name="v", bufs=1))
        pt_pool = ep(tc.tile_pool(name="pt", bufs=4))
        at_pool = ep(tc.tile_pool(name="at", bufs=6))
        nrm_pool = ep(tc.tile_pool(name="nrm", bufs=2))
        out_pool = ep(tc.tile_pool(name="outp", bufs=3))
        misc_pool = ep(tc.tile_pool(name="misc", bufs=1))
        if variant == "general":
            mk_pool = ep(tc.tile_pool(name="mk", bufs=4))

        ps_mm = ep(tc.tile_pool(name="ps_mm", bufs=2, space="PSUM"))
        ps_sc = ep(tc.tile_pool(name="ps_sc", bufs=3, space="PSUM"))
        ps_pv = ep(tc.tile_pool(name="ps_pv", bufs=3, space="PSUM"))

        # constants
        tri_sb = misc_pool.tile([128, 128], f32, name="tri_sb")
        nc.gpsimd.dma_start(tri_sb[:], tri[:])
        ones1_sb = misc_pool.tile([1, 64], f32, name="ones1_sb")
        nc.gpsimd.memset(ones1_sb[:], 1.0)

        # ---- phase 1: projections ----
        qT_sb = [qT_pool.tile([128, S], f32, name=f"qT{d}") for d in range(4)]
        kT_sb = [kT_pool.tile([128, S], f32, name=f"kT{d}") for d in range(4)]
        v_sb = [v_pool.tile([128, HPG * 65], f32, name=f"v{t}") for t in range(NC)]

        for t in range(NC):
            # ones columns for the softmax denominator
            v3 = v_sb[t].rearrange("p (h c) -> p h c", h=HPG, c=65)
            nc.gpsimd.memset(v3[:, :, 64:65], 1.0)

        for j in range(NJ):
            for w_dram, x_dram, kind in (
                    (wq, xqT, "q"), (wk, xkT, "k"), (wv, xvT, "v")):
                w_ch = []
                for mc in range(MK):
                    wt = w_pool.tile([128, DH], f32, name=f"w_{kind}{j}_{mc}", tag="w")
                    nc.gpsimd.dma_start(wt[:], w_dram[mc * 128:(mc + 1) * 128, :])
                    w_ch.append(wt)
                x_ch = []
                for mc in range(MK):
                    xt = x_pool.tile([128, SB], f32, name=f"x_{kind}{j}_{mc}", tag="x")
                    nc.gpsimd.dma_start(
                        xt[:], x_dram[mc * 128:(mc + 1) * 128, j * SB:(j + 1) * SB])
                    x_ch.append(xt)
                if kind in ("q", "k"):
                    dst = qT_sb if kind == "q" else kT_sb
                    for d in range(4):
                        ps = ps_mm.tile([128, SB], f32, name=f"ps_{kind}{j}_{d}", tag="mm")
                        for mc in range(MK):
                            nc.tensor.matmul(
                                ps[:], w_ch[mc][:, d * 128:(d + 1) * 128], x_ch[mc][:],
                                start=(mc == 0), stop=(mc == MK - 1))
                        nc.vector.tensor_copy(dst[d][:, j * SB:(j + 1) * SB], ps[:])
                else:
                    for st in range(4):
                        t = 4 * j + st
                        ps = ps_mm.tile([128, DH], f32, name=f"ps_v{t}", tag="mm")
                        for mc in range(MK):
                            nc.tensor.matmul(
                                ps[:], x_ch[mc][:, st * 128:(st + 1) * 128], w_ch[mc][:],
                                start=(mc == 0), stop=(mc == MK - 1))
                        v3 = v_sb[t].rearrange("p (h c) -> p h c", h=HPG, c=65)
                        p3 = ps.rearrange("p (h c) -> p h c", h=HPG, c=64)
                        nc.vector.tensor_copy(v3[:, :, 0:64], p3[:])

        wo_sb = []
        for d in range(4):
            wt = wo_pool.tile([128, M], f32, name=f"wo{d}")
            nc.gpsimd.dma_start(wt[:], wo[d * 128:(d + 1) * 128, :])
            wo_sb.append(wt)

        # ---- phase 2: attention + output projection, per sq block ----
        def emit_score_chunk(j, h, c):
            """scoresT chunk -> exp -> PT tile; returns pt tile."""
            dtile, drow = h // 2, 64 * (h % 2)
            sc = ps_sc.tile([128, SB], f32, name=f"sc{j}_{h}_{c}", tag="sc")
            nc.tensor.matmul(
                sc[:],
                kT_sb[dtile][drow:drow + 64, c * CK:(c + 1) * CK],
                qT_sb[dtile][drow:drow + 64, j * SB:(j + 1) * SB],
                start=True, stop=True)
            pt = pt_pool.tile([128, SB], f32, name=f"pt{j}_{h}_{c}", tag="pt")
            if variant == "causal" and c >= 4 * j:
                o = 128 * (c - 4 * j)
                nc.vector.tensor_add(
                    sc[:, o:o + 128], sc[:, o:o + 128], tri_sb[:])
                if o > 0:
                    nc.gpsimd.memset(pt[:, 0:o], 0.0)
                nc.scalar.activation(
                    pt[:, o:SB], sc[:, o:SB],
                    mybir.ActivationFunctionType.Exp, scale=0.125)
            else:
                if variant == "general":
                    mk = mk_pool.tile([128, SB], f32, name=f"mk{j}_{h}_{c}", tag="mk")
                    nc.gpsimd.dma_start(
                        mk[:], maskT[c * CK:(c + 1) * CK, j * SB:(j + 1) * SB])
                    nc.vector.tensor_add(sc[:], sc[:], mk[:])
                nc.scalar.activation(
                    pt[:], sc[:], mybir.ActivationFunctionType.Exp, scale=0.125)
            return pt

        def emit_normalize(j, h, pv, at):
            """divide pv rows 0:64 by denominator row 64 into at slice."""
            drow = 64 * (h % 2)
            dn = nrm_pool.tile([1, SB], f32, name=f"dn{j}_{h}", tag="dn")
            nc.vector.tensor_copy(dn[:], pv[64:65, :])
            dnr = nrm_pool.tile([1, SB], f32, name=f"dnr{j}_{h}", tag="dnr")
            nc.vector.reciprocal(dnr[:], dn[:])
            rb = ps_mm.tile([64, SB], f32, name=f"rb{j}_{h}", tag="mm")
            nc.tensor.matmul(rb[:], ones1_sb[:], dnr[:], start=True, stop=True)
            rb_sb = nrm_pool.tile([64, SB], f32, name=f"rbs{j}_{h}", tag="rbs")
            nc.scalar.copy(rb_sb[:], rb[:])
            nc.vector.tensor_mul(at[drow:drow + 64, :], pv[0:64, :], rb_sb[:])

        for j in range(NJ):
            nchunks = 4 * (j + 1) if variant == "causal" else NC
            at_tiles = []
            for hp in range(HPG // 2):
                hA, hB = 2 * hp, 2 * hp + 1
                at = at_pool.tile([128, SB], f32, name=f"at{j}_{hp}", tag="at")
                at_tiles.append(at)
                pvA = ps_pv.tile([65, SB], f32, name=f"pv{j}_{hA}", tag="pv")
                pvB = ps_pv.tile([65, SB], f32, name=f"pv{j}_{hB}", tag="pv")
                for c in range(nchunks):
                    ptA = emit_score_chunk(j, hA, c)
                    ptB = emit_score_chunk(j, hB, c)
                    nc.tensor.matmul(
                        pvA[:], v_sb[c][:, 65 * hA:65 * hA + 65], ptA[:],
                        start=(c == 0), stop=(c == nchunks - 1))
                    nc.tensor.matmul(
                        pvB[:], v_sb[c][:, 65 * hB:65 * hB + 65], ptB[:],
                        start=(c == 0), stop=(c == nchunks - 1))
                emit_normalize(j, hA, pvA, at)
                emit_normalize(j, hB, pvB, at)

            for ss in range(4):
                for nh in range(2):
                    ps = ps_mm.tile([128, SB], f32, name=f"po{j}_{ss}_{nh}", tag="mm")
                    for d in range(4):
                        nc.tensor.matmul(
                            ps[:],
                            at_tiles[d][:, ss * 128:(ss + 1) * 128],
                            wo_sb[d][:, nh * SB:(nh + 1) * SB],
                            start=(d == 0), stop=(d == 3))
                    ot = out_pool.tile([128, SB], f32, name=f"ot{j}_{ss}_{nh}", tag="ot")
                    nc.vector.tensor_copy(ot[:], ps[:])
                    r0 = j * SB + ss * 128
                    nc.gpsimd.dma_start(
                        out[r0:r0 + 128, nh * SB:(nh + 1) * SB], ot[:])

    nc.compile()
    return nc


def _get_program(variant):
    if variant not in _PROG_CACHE:
        _PROG_CACHE[variant] = _build_program(variant)
    return _PROG_CACHE[variant]


def _host_prep(queries, keys, values, masks, Wq, Wk, Wv):
    """Build the 8 per-core input maps."""
    tril = np.tril(np.ones((S, S), dtype=bool))
    if all(np.array_equal(masks[b], tril) for b in range(B)):
        variant = "causal"
    elif masks.all():
        variant = "allones"
    else:
        variant = "general"

    sq = np.arange(128)
    tri_np = np.where(sq[None, :] >= sq[:, None], 0.0, -1.0e6).astype(np.float32)

    # [H, M, D] -> [M, H*D] head-major per group
    def wcat(w, g):
        return np.ascontiguousarray(
            w[g * HPG:(g + 1) * HPG].transpose(1, 0, 2).reshape(M, DH))

    in_maps = []
    for c in range(NCORES):
        b, g = c // G, c % G
        m = {
            "xqT": np.ascontiguousarray(queries[b].T),
            "xkT": np.ascontiguousarray(keys[b].T),
            "xvT": np.ascontiguousarray(values[b].T),
            "wq": wcat(Wq, g),
            "wk": wcat(Wk, g),
            "wv": wcat(Wv, g),
            "tri": tri_np,
        }
        if variant == "general":
            m["maskT"] = np.where(masks[b].T, 0.0, -1.0e6).astype(np.float32)
        in_maps.append(m)
    return variant, in_maps


def run(queries, keys, values, masks, Wq, Wk, Wv, Wo, bo, trace=False):
    from concourse import bass_utils

    queries = np.asarray(queries, np.float32)
    keys = np.asarray(keys, np.float32)
    values = np.asarray(values, np.float32)
    masks = np.asarray(masks, bool)
    Wq = np.asarray(Wq, np.float32)
    Wk = np.asarray(Wk, np.float32)
    Wv = np.asarray(Wv, np.float32)
    Wo = np.asarray(Wo, np.float32)
    bo = np.asarray(bo, np.float32)

    variant, in_maps = _host_prep(queries, keys, values, masks, Wq, Wk, Wv)
    for c in range(NCORES):
        g = c % G
        in_maps[c]["wo"] = np.ascontiguousarray(Wo[g * DH:(g + 1) * DH, :])

    nc = _get_program(variant)
    res = bass_utils.run_bass_kernel_spmd(
        nc, in_maps, list(range(NCORES)), trace=trace)

    out = np.empty((B, S, M), np.float32)
    for b in range(B):
        out[b] = res.results[G * b]["out"] + res.results[G * b + 1]["out"] + bo
    return out, res


def kernel(queries, keys, values, masks, Wq, Wk, Wv, Wo, bo):
    out, _ = run(queries, keys, values, masks, Wq, Wk, Wv, Wo, bo, trace=False)
    return out


# revision 12
# speedup vs baseline: 2.2034x; 2.1068x over previous
"""Multi-head attention (B=4, S=2048, M=1024, H=16, D=64) on 8 trn2 cores.

Sharding: core c = (b, g) with b = c // 2 (batch), g = c % 2 (head group of 8
heads).  Each core computes q/k/v projections for its 8 heads, causal
attention, and a partial output projection (contraction over its 512 feature
rows of Wo).  Host sums the two partials per batch and adds the bias.

Device-side layouts (all fp32):
  xqT/xkT/xvT  [1024(m), 2048(s)]   host-transposed activations
  wq/wk/wv     [1024(m), 512(dh)]   dh = 64*h_local + d  (head-major)
  wo           [512(dh), 1024(n)]
  qT,kT        [512(dh), 2048(s)]   = (X W)^T, computed as W.T @ X.T
  v            [2048(s), 520]       per 128-row tile: cols 65h..65h+63 = v_h,
                                    col 65h+64 = 1.0 (softmax denominator)
  scoresT      [sk, sq]             = kT_h.T @ qT_h  (contraction over d=64)
  PT           exp((scoresT + mask)/8)  -- no max subtraction (|scores/8|<~2)
  outT_h       [65, sq]             = [v_h|1].T @ PT ; row 64 = sum_sk PT
  attnT        [512(dh), sq]        = outT_h / denom, heads stacked
  out_partial  [2048(s), 1024(n)]   = attnT.T @ wo   (no bias)
"""

import os
import sys

for _p in ("/opt/trn_rl_repo", "/root/.axon_site/_ro/trn_rl_repo"):
    if os.path.isdir(_p) and _p not in sys.path:
        sys.path.append(_p)

import numpy as np

B, S, M, H, D = 4, 2048, 1024, 16, 64
G = 2               # head groups (cores per batch)
HPG = H // G        # heads per group = 8
DH = HPG * D        # feature rows per group = 512
NCORES = B * G
SB = 512            # sq block (matmul N)
CK = 128            # sk chunk (matmul M / partition)
NJ = S // SB        # 4 sq blocks
NC = S // CK        # 16 sk chunks
MK = M // 128       # 8 m chunks

_PROG_CACHE = {}


def _build_program(variant):
    """variant: 'causal' | 'allones' | 'general'"""
    import concourse.bass as bass
    import concourse.bacc as bacc
    import concourse.mybir as mybir
    from concourse import tile
    from contextlib import ExitStack

    f32 = mybir.dt.float32
    f32r = mybir.dt.float32r
    nc = bacc.Bacc("TRN2", target_bir_lowering=False, debug=False, num_devices=NCORES)

    xqT = nc.dram_tensor("xqT", [M, S], f32, kind="ExternalInput").ap()
    xkT = nc.dram_tensor("xkT", [M, S], f32, kind="ExternalInput").ap()
    xvT = nc.dram_tensor("xvT", [M, S], f32, kind="ExternalInput").ap()
    wq = nc.dram_tensor("wq", [M, DH], f32, kind="ExternalInput").ap()
    wk = nc.dram_tensor("wk", [M, DH], f32, kind="ExternalInput").ap()
    wv = nc.dram_tensor("wv", [M, DH], f32, kind="ExternalInput").ap()
    wo = nc.dram_tensor("wo", [DH, M], f32, kind="ExternalInput").ap()
    tri = nc.dram_tensor("tri", [128, 128], f32, kind="ExternalInput").ap()
    ind8 = nc.dram_tensor("ind8", [8, SB], f32, kind="ExternalInput").ap()
    if variant == "general":
        maskT = nc.dram_tensor("maskT", [S, S], f32, kind="ExternalInput").ap()
    out = nc.dram_tensor("out", [S, M], f32, kind="ExternalOutput").ap()

    with tile.TileContext(nc) as tc, ExitStack() as ctx:
        ep = ctx.enter_context
        ctx.enter_context(nc.allow_low_precision(reason="f32r matmul inputs"))
        dma = nc.sync.dma_start

        w_pool = ep(tc.tile_pool(name="w", bufs=9))
        x_pool = ep(tc.tile_pool(name="x", bufs=10))
        wo_pool = ep(tc.tile_pool(name="wo", bufs=1))
        qT_pool = ep(tc.tile_pool(name="qT", bufs=1))
        kT_pool = ep(tc.tile_pool(name="kT", bufs=1))
        v_pool = ep(tc.tile_pool(name="v", bufs=1))
        pt_pool = ep(tc.tile_pool(name="pt", bufs=6))
        at_pool = ep(tc.tile_pool(name="at", bufs=6))
        nrm_pool = ep(tc.tile_pool(name="nrm", bufs=3))
        out_pool = ep(tc.tile_pool(name="outp", bufs=3))
        misc_pool = ep(tc.tile_pool(name="misc", bufs=1))
        if variant == "general":
            mk_pool = ep(tc.tile_pool(name="mk", bufs=4))

        ps_mm = ep(tc.tile_pool(name="ps_mm", bufs=2, space="PSUM"))
        ps_sc = ep(tc.tile_pool(name="ps_sc", bufs=3, space="PSUM"))
        ps_pv = ep(tc.tile_pool(name="ps_pv", bufs=3, space="PSUM"))

        # constants
        tri_sb = misc_pool.tile([128, 128], f32, name="tri_sb")
        dma(tri_sb[:], tri[:])
        ind8_sb = misc_pool.tile([8, SB], f32r, name="ind8_sb")
        dma(ind8_sb[:], ind8[:].bitcast(f32r))

        qT_sb = [qT_pool.tile([128, S], f32r, name=f"qT{d}") for d in range(4)]
        kT_sb = [kT_pool.tile([128, S], f32r, name=f"kT{d}") for d in range(4)]
        v_sb = [v_pool.tile([128, HPG * 65], f32r, name=f"v{t}") for t in range(NC)]

        for t in range(NC):
            v3 = v_sb[t].bitcast(f32).rearrange("p (h c) -> p h c", h=HPG, c=65)
            nc.gpsimd.memset(v3[:, :, 64:65], 1.0)

        # ---- phase 1: projections, per s-block ----
        for j in range(NJ):
            for w_dram, x_dram, kind in (
                    (wq, xqT, "q"), (wk, xkT, "k"), (wv, xvT, "v")):
                w_ch = []
                for mc in range(MK):
                    wt = w_pool.tile([128, DH], f32r, name=f"w_{kind}{j}_{mc}", tag="w")
                    dma(wt[:], w_dram[mc * 128:(mc + 1) * 128, :].bitcast(f32r))
                    w_ch.append(wt)
                x_ch = []
                for mc in range(MK):
                    xt = x_pool.tile([128, SB], f32r, name=f"x_{kind}{j}_{mc}", tag="x")
                    dma(xt[:], x_dram[mc * 128:(mc + 1) * 128, j * SB:(j + 1) * SB].bitcast(f32r))
                    x_ch.append(xt)
                if kind in ("q", "k"):
                    dst = qT_sb if kind == "q" else kT_sb
                    for d in range(4):
                        ps = ps_mm.tile([128, SB], f32, name=f"ps_{kind}{j}_{d}", tag="mm")
                        for mc in range(MK):
                            nc.tensor.matmul(
                                ps[:], w_ch[mc][:, d * 128:(d + 1) * 128], x_ch[mc][:],
                                start=(mc == 0), stop=(mc == MK - 1))
                        nc.vector.tensor_copy(dst[d][:, j * SB:(j + 1) * SB], ps[:])
                else:
                    for st in range(4):
                        t = 4 * j + st
                        ps = ps_mm.tile([128, DH], f32, name=f"ps_v{t}", tag="mm")
                        for mc in range(MK):
                            nc.tensor.matmul(
                                ps[:], x_ch[mc][:, st * 128:(st + 1) * 128], w_ch[mc][:],
                                start=(mc == 0), stop=(mc == MK - 1))
                        v3 = v_sb[t].rearrange("p (h c) -> p h c", h=HPG, c=65)
                        p3 = ps.rearrange("p (h c) -> p h c", h=HPG, c=64)
                        nc.vector.tensor_copy(v3[:, :, 0:64], p3[:])

        wo_sb = []
        for d in range(4):
            wt = wo_pool.tile([128, M], f32r, name=f"wo{d}")
            dma(wt[:], wo[d * 128:(d + 1) * 128, :].bitcast(f32r))
            wo_sb.append(wt)

        # ---- phase 2: attention + output projection, per sq block ----
        def emit_score_chunk(j, h, c):
            """scoresT chunk -> exp -> PT tile; returns (pt, o)."""
            dtile, drow = h // 2, 64 * (h % 2)
            sc = ps_sc.tile([128, SB], f32, name=f"sc{j}_{h}_{c}", tag="sc")
            nc.tensor.matmul(
                sc[:],
                kT_sb[dtile][drow:drow + 64, c * CK:(c + 1) * CK],
                qT_sb[dtile][drow:drow + 64, j * SB:(j + 1) * SB],
                start=True, stop=True)
            pt = pt_pool.tile([128, SB], f32r, name=f"pt{j}_{h}_{c}", tag="pt")
            o = 0
            if variant == "causal" and c >= 4 * j:
                o = 128 * (c - 4 * j)
                nc.vector.tensor_add(
                    sc[:, o:o + 128], sc[:, o:o + 128], tri_sb[:])
            elif variant == "general":
                mk = mk_pool.tile([128, SB], f32, name=f"mk{j}_{h}_{c}", tag="mk")
                dma(mk[:], maskT[c * CK:(c + 1) * CK, j * SB:(j + 1) * SB])
                nc.vector.tensor_add(sc[:], sc[:], mk[:])
            nc.scalar.activation(
                pt[:, o:SB], sc[:, o:SB],
                mybir.ActivationFunctionType.Exp, scale=0.125)
            return pt, o

        for j in range(NJ):
            nchunks = 4 * (j + 1) if variant == "causal" else NC
            at_tiles = []
            dn_all = nrm_pool.tile([8, SB], f32r, name=f"dn{j}", tag="dn")
            for hp in range(HPG // 2):
                hA, hB = 2 * hp, 2 * hp + 1
                at = at_pool.tile([128, SB], f32r, name=f"at{j}_{hp}", tag="at")
                at_tiles.append(at)
                pvA = ps_pv.tile([65, SB], f32, name=f"pv{j}_{hA}", tag="pv")
                pvB = ps_pv.tile([65, SB], f32, name=f"pv{j}_{hB}", tag="pv")
                for c in range(nchunks):
                    ptA, oA = emit_score_chunk(j, hA, c)
                    ptB, oB = emit_score_chunk(j, hB, c)
                    nc.tensor.matmul(
                        pvA[:, oA:SB], v_sb[c][:, 65 * hA:65 * hA + 65], ptA[:, oA:SB],
                        start=(c == 0), stop=(c == nchunks - 1))
                    nc.tensor.matmul(
                        pvB[:, oB:SB], v_sb[c][:, 65 * hB:65 * hB + 65], ptB[:, oB:SB],
                        start=(c == 0), stop=(c == nchunks - 1))
                for h, pv in ((hA, pvA), (hB, pvB)):
                    drow = 64 * (h % 2)
                    nc.vector.tensor_copy(at[drow:drow + 64, :], pv[0:64, :])
                    dnt = nrm_pool.tile([1, SB], f32r, name=f"dnt{j}_{h}", tag="dnt")
                    nc.scalar.copy(dnt[:], pv[64:65, :])
                    dma(dn_all[h:h + 1, :], dnt[:])
            dnr = nrm_pool.tile([8, SB], f32r, name=f"dnr{j}", tag="dnr")
            nc.vector.reciprocal(dnr[:], dn_all[:])
            for hp in range(HPG // 2):
                rb = ps_mm.tile([128, SB], f32, name=f"rb{j}_{hp}", tag="mm")
                nc.tensor.matmul(
                    rb[:], ind8_sb[:, hp * 128:(hp + 1) * 128], dnr[:],
                    start=True, stop=True)
                rb_sb = nrm_pool.tile([128, SB], f32r, name=f"rbs{j}_{hp}", tag="rbs")
                nc.scalar.copy(rb_sb[:], rb[:])
                nc.vector.tensor_mul(at_tiles[hp][:], at_tiles[hp][:], rb_sb[:])

            for ss in range(4):
                for nh in range(2):
                    ps = ps_mm.tile([128, SB], f32, name=f"po{j}_{ss}_{nh}", tag="mm")
                    for d in range(4):
                        nc.tensor.matmul(
                            ps[:],
                            at_tiles[d][:, ss * 128:(ss + 1) * 128],
                            wo_sb[d][:, nh * SB:(nh + 1) * SB],
                            start=(d == 0), stop=(d == 3))
                    ot = out_pool.tile([128, SB], f32, name=f"ot{j}_{ss}_{nh}", tag="ot")
                    nc.vector.tensor_copy(ot[:], ps[:])
                    r0 = j * SB + ss * 128
                    dma(out[r0:r0 + 128, nh * SB:(nh + 1) * SB], ot[:])

    nc.compile()
    return nc


def _get_program(variant):
    if variant not in _PROG_CACHE:
        _PROG_CACHE[variant] = _build_program(variant)
    return _PROG_CACHE[variant]


def _host_prep(queries, keys, values, masks, Wq, Wk, Wv):
    """Build the 8 per-core input maps."""
    tril = np.tril(np.ones((S, S), dtype=bool))
    if all(np.array_equal(masks[b], tril) for b in range(B)):
        variant = "causal"
    elif masks.all():
        variant = "allones"
    else:
        variant = "general"

    sq = np.arange(128)
    tri_np = np.where(sq[None, :] >= sq[:, None], 0.0, -1.0e6).astype(np.float32)
    ind8_np = np.zeros((8, 512), np.float32)
    for c in range(4):
        for cc in range(128):
            ind8_np[2 * c + cc // 64, 128 * c + cc] = 1.0

    # [H, M, D] -> [M, H*D] head-major per group
    def wcat(w, g):
        return np.ascontiguousarray(
            w[g * HPG:(g + 1) * HPG].transpose(1, 0, 2).reshape(M, DH))

    in_maps = []
    for c in range(NCORES):
        b, g = c // G, c % G
        m = {
            "xqT": np.ascontiguousarray(queries[b].T),
            "xkT": np.ascontiguousarray(keys[b].T),
            "xvT": np.ascontiguousarray(values[b].T),
            "wq": wcat(Wq, g),
            "wk": wcat(Wk, g),
            "wv": wcat(Wv, g),
            "tri": tri_np,
            "ind8": ind8_np,
        }
        if variant == "general":
            m["maskT"] = np.where(masks[b].T, 0.0, -1.0e6).astype(np.float32)
        in_maps.append(m)
    return variant, in_maps


def run(queries, keys, values, masks, Wq, Wk, Wv, Wo, bo, trace=False):
    from concourse import bass_utils

    queries = np.asarray(queries, np.float32)
    keys = np.asarray(keys, np.float32)
    values = np.asarray(values, np.float32)
    masks = np.asarray(masks, bool)
    Wq = np.asarray(Wq, np.float32)
    Wk = np.asarray(Wk, np.float32)
    Wv = np.asarray(Wv, np.float32)
    Wo = np.asarray(Wo, np.float32)
    bo = np.asarray(bo, np.float32)

    variant, in_maps = _host_prep(queries, keys, values, masks, Wq, Wk, Wv)
    for c in range(NCORES):
        g = c % G
        in_maps[c]["wo"] = np.ascontiguousarray(Wo[g * DH:(g + 1) * DH, :])

    nc = _get_program(variant)
    res = bass_utils.run_bass_kernel_spmd(
        nc, in_maps, list(range(NCORES)), trace=trace)

    out = np.empty((B, S, M), np.float32)
    for b in range(B):
        out[b] = res.results[G * b]["out"] + res.results[G * b + 1]["out"] + bo
    return out, res


def kernel(queries, keys, values, masks, Wq, Wk, Wv, Wo, bo):
    out, _ = run(queries, keys, values, masks, Wq, Wk, Wv, Wo, bo, trace=False)
    return out
